# revision 1
# baseline (speedup 1.0000x reference)
"""Trainium2 Bass kernel for nn_Encoder_17918603559377 (4-layer sparse-attention
encoder, top-16 per row, B=2 S=1024 D=512 H=8).

Sharding: 8 cores; core c handles batch c//4, heads {2r, 2r+1} where r = c%4
(tensor-parallel over heads within each batch group of 4 cores). Per layer the
partial attention outputs are AllReduce-summed within each group of 4; the
residual + LayerNorm is computed redundantly on every core.

All matmuls run in true fp32 (the top-16 threshold is sensitive to score
precision). Top-16 per score row is computed exactly with vector.max (top-8) +
match_replace + vector.max, and the masked softmax numerator + row sum come
from a fused custom DVE op (select(e >= e16, e, 0) with ADD accumulation).
"""

import sys

sys.path.insert(0, "/opt/trn_rl_repo")

import numpy as np

L, B, S, D, H, DK = 4, 2, 1024, 512, 8, 64
TOPK = 16
EPS = 1e-6
SCALE = 1.0 / np.sqrt(DK)
NT = S // 128  # token tiles per batch
NDT = D // 128  # d-dim tiles

_COMPILED = None


def _register_sel_op():
    """p = select(e >= e16, e * invZ, 0) in one DVE pass (s0=e16, s1=invZ)."""
    from concourse.dve_ops import DveOp, OPS
    import concourse.dve_ops as dops
    from concourse.dve_spec import Spec, Src0, C0, C1, Zero, select, lower
    from concourse.dve_uop import DveOpSpec

    for op in OPS:
        if op.name == "SELSC_GE_ANT":
            return op
    spec = Spec(
        body=select(Src0 >= C0, Src0 * C1, Zero),
        reference=lambda in0, in1, s0, s1, imm2: np.where(in0 >= s0, in0 * s1, 0.0),
    )
    op = DveOp("SELSC_GE_ANT", spec, subdim=False, uops_sha={})
    OPS.append(op)
    dops._SUB_OPCODE_FOR_NAME[op.name] = dops._CUSTOM_DVE_ROW_BASE + len(OPS) - 1
    for ver in ("v3", "v4"):
        tmp = DveOpSpec(
            name=op.name,
            opcode=dops.get_dve_sub_opcode(op.name),
            uops=lower(spec, ver=ver),
            rd1_en=False,
        )
        op.uops_sha[ver] = tmp.sha(ver)
    return op


def _build(reps=1, sim=False):
    import concourse.bacc as bacc
    import concourse.mybir as mybir
    import concourse.tile as tile
    from concourse import masks

    SEL = _register_sel_op()
    f32 = mybir.dt.float32
    AL = mybir.AluOpType
    AF = mybir.ActivationFunctionType

    nc = bacc.Bacc(
        "TRN2", target_bir_lowering=False, debug=False,
        num_devices=(1 if sim else 8),
    )

    x_d = nc.dram_tensor("x", (S, D), f32, kind="ExternalInput")
    wq_d = nc.dram_tensor("wq", (L, D, 128), f32, kind="ExternalInput")
    wk_d = nc.dram_tensor("wk", (L, D, 128), f32, kind="ExternalInput")
    wv_d = nc.dram_tensor("wv", (L, D, 128), f32, kind="ExternalInput")
    wo_d = nc.dram_tensor("wo", (L, 128, D), f32, kind="ExternalInput")
    bq_d = nc.dram_tensor("bq", (L, 128), f32, kind="ExternalInput")
    bk_d = nc.dram_tensor("bk", (L, 128), f32, kind="ExternalInput")
    rows_d = nc.dram_tensor("rows", (3 * L, D), f32, kind="ExternalInput")
    # rows: [0:L] beta, [L:2L] gamma, [2L:3L] B[l]/4
    # where B[l] = bv[l] @ Wo[l] + bo[l] (v-bias and o-bias folded, added to the
    # per-core y partial via a rank-1 matmul; the 4-way AllReduce restores B)
    out_d = nc.dram_tensor("out", (S, D), f32, kind="ExternalOutput")

    cc_in = [
        nc.dram_tensor(f"cc_in{l}", (S, D), f32, kind="Internal") for l in range(L)
    ]
    cc_out = [
        nc.dram_tensor(f"cc_out{l}", (S, D), f32, kind="Internal") for l in range(L)
    ]
    GROUPS = [[0, 1, 2, 3], [4, 5, 6, 7]]

    with tile.TileContext(nc) as tc:
        with (
            tc.tile_pool(name="w", bufs=1) as wp,
            tc.tile_pool(name="state", bufs=1) as st,
            tc.tile_pool(name="sb", bufs=2) as sb,
            tc.tile_pool(name="sm", bufs=3) as sm,
            tc.tile_pool(name="ps_s", bufs=2, space="PSUM") as ps_s,
            tc.tile_pool(name="ps_w", bufs=2, space="PSUM") as ps_w,
            tc.tile_pool(name="ps_o", bufs=1, space="PSUM") as ps_o,
        ):
            ident = wp.tile([128, 128], f32, tag="ident")
            masks.make_identity(nc, ident[:])

            # --- weight preload (all layers) ---
            wq_sb = wp.tile([128, L, NDT, 128], f32, tag="wq")
            wk_sb = wp.tile([128, L, NDT, 128], f32, tag="wk")
            wv_sb = wp.tile([128, L, NDT, 128], f32, tag="wv")
            wo_sb = wp.tile([128, L, D], f32, tag="wo")
            for l in range(L):
                for w_sb, w_d in ((wq_sb, wq_d), (wk_sb, wk_d), (wv_sb, wv_d)):
                    nc.sync.dma_start(
                        w_sb[:, l],
                        w_d[l].rearrange("(kc p) m -> p kc m", p=128),
                    )
                nc.sync.dma_start(wo_sb[:, l], wo_d[l])
            bq_sb = [wp.tile([1, 128], f32, name=f"bqs{l}", tag=f"bq{l}") for l in range(L)]
            for l in range(L):
                nc.sync.dma_start(bq_sb[l][:], bq_d[l : l + 1, :])
            ones_row = wp.tile([1, S], f32, tag="ones_row")
            nc.vector.memset(ones_row[:], 1.0)

            # broadcast rows (beta, gamma', B0) to [128, D]
            rows_row = wp.tile([1, 3 * L, D], f32, tag="rows_row")
            nc.sync.dma_start(rows_row[:], rows_d[:].rearrange("(o r) d -> o r d", o=1))
            rows_bc = wp.tile([128, 2 * L, D], f32, tag="rows_bc")
            for r in range(2 * L):
                nc.gpsimd.partition_broadcast(rows_bc[:, r], rows_row[:, r])

            # --- state ---
            h_sb = st.tile([128, NT, D], f32, tag="h")  # token-major h
            for _rep in range(reps):
                nc.sync.dma_start(h_sb[:], x_d[:].rearrange("(c p) d -> p c d", p=128))
                hT_sb = st.tile([128, NDT, S], f32, tag="hT", name=f"hT{_rep}")
                qT_sb = st.tile([128, S], f32, tag="qT", name=f"qT{_rep}")
                kT_sb = st.tile([128, S], f32, tag="kT", name=f"kT{_rep}")
                v_sb = st.tile([128, NT, 128], f32, tag="v", name=f"v{_rep}")
                oT_sb = st.tile([128, S], f32, tag="oT", name=f"oT{_rep}")

                for l in range(L):
                    oT_ps = ps_o.tile([128, S], f32, tag="oT")
                    # ---- transpose h -> hT ----
                    for dt in range(NDT):
                        tp = ps_w.tile([128, S], f32, tag="work")
                        for c in range(NT):
                            nc.tensor.transpose(
                                tp[:, c * 128 : (c + 1) * 128],
                                h_sb[:, c, dt * 128 : (dt + 1) * 128],
                                ident[:],
                            )
                        nc.scalar.copy(hT_sb[:, dt], tp[:])

                    # ---- qT, kT ----
                    qT_ps = ps_w.tile([128, S], f32, tag="work", name=f"qTps{_rep}_{l}")
                    for dt in range(NDT):
                        for nh in range(2):
                            nc.tensor.matmul(
                                qT_ps[:, nh * 512 : (nh + 1) * 512],
                                wq_sb[:, l, dt],
                                hT_sb[:, dt, nh * 512 : (nh + 1) * 512],
                                start=(dt == 0),
                                stop=False,
                            )
                    for nh in range(2):
                        nc.tensor.matmul(
                            qT_ps[:, nh * 512 : (nh + 1) * 512],
                            bq_sb[l][:],
                            ones_row[:, nh * 512 : (nh + 1) * 512],
                            start=False,
                            stop=True,
                        )
                    nc.scalar.copy(qT_sb[:], qT_ps[:])
                    kT_ps = ps_w.tile([128, S], f32, tag="work", name=f"kTps{_rep}_{l}")
                    for dt in range(NDT):
                        for nh in range(2):
                            nc.tensor.matmul(
                                kT_ps[:, nh * 512 : (nh + 1) * 512],
                                wk_sb[:, l, dt],
                                hT_sb[:, dt, nh * 512 : (nh + 1) * 512],
                                start=(dt == 0),
                                stop=(dt == NDT - 1),
                            )
                    nc.scalar.copy(kT_sb[:], kT_ps[:])
                    # ---- v (no bias; folded into B) ----
                    v_ps = ps_w.tile([128, S], f32, tag="work")
                    for c in range(NT):
                        for dt in range(NDT):
                            nc.tensor.matmul(
                                v_ps[:, c * 128 : (c + 1) * 128],
                                hT_sb[:, dt, c * 128 : (c + 1) * 128],
                                wv_sb[:, l, dt],
                                start=(dt == 0),
                                stop=(dt == NDT - 1),
                            )
                    nc.scalar.copy(v_sb[:], v_ps[:])

                    # ---- attention per (head, q-tile) ----
                    for h in range(2):
                        hs = slice(h * 64, (h + 1) * 64)
                        for qt in range(NT):
                            e = sb.tile([128, S], f32, tag="e")
                            for nh in range(2):
                                s_ps = ps_s.tile(
                                    [128, 512], f32, tag="s", name=f"sps{_rep}_{l}_{h}_{qt}_{nh}"
                                )
                                nc.tensor.matmul(
                                    s_ps[:],
                                    qT_sb[hs, qt * 128 : (qt + 1) * 128],
                                    kT_sb[hs, nh * 512 : (nh + 1) * 512],
                                    start=True,
                                    stop=True,
                                )
                                nc.scalar.activation(
                                    e[:, nh * 512 : (nh + 1) * 512],
                                    s_ps[:],
                                    AF.Exp,
                                    scale=float(SCALE),
                                )
                            m8a = sm.tile([128, 8], f32, tag="m8a")
                            m8b = sm.tile([128, 8], f32, tag="m8b")
                            e2 = sb.tile([128, S], f32, tag="e2")
                            nc.vector.max(m8a[:], e[:])
                            nc.vector.match_replace(e2[:], m8a[:], e[:], 0.0)
                            nc.vector.max(m8b[:], e2[:])
                            e16 = sm.tile([128, 1], f32, tag="e16")
                            nc.vector.tensor_copy(e16[:], m8b[:, 7:8])
                            # Z = sum of the 16 kept exp values (= m8a + m8b sums)
                            dm = sm.tile([128, 8], f32, tag="dm")
                            za = sm.tile([128, 1], f32, tag="za")
                            nc.scalar.activation(dm[:], m8a[:], AF.Copy, accum_out=za[:])
                            dmb = sm.tile([128, 8], f32, tag="dmb")
                            zb = sm.tile([128, 1], f32, tag="zb")
                            nc.scalar.activation(dmb[:], m8b[:], AF.Copy, accum_out=zb[:])
                            zs = sm.tile([128, 1], f32, tag="zs")
                            nc.vector.tensor_add(zs[:], za[:], zb[:])
                            iz = sm.tile([128, 1], f32, tag="iz")
                            nc.vector.reciprocal(iz[:], zs[:])
                            p = sb.tile([128, S], f32, tag="p")
                            nc.vector._custom_dve(
                                SEL, out=p[:], in0=e[:], s0=e16[:], s1=iz[:]
                            )
                            pT_ps = ps_w.tile([128, S], f32, tag="work")
                            for kc in range(NT):
                                nc.tensor.transpose(
                                    pT_ps[:, kc * 128 : (kc + 1) * 128],
                                    p[:, kc * 128 : (kc + 1) * 128],
                                    ident[:],
                                )
                            pT = sb.tile([128, NT, 128], f32, tag="pT")
                            nc.scalar.copy(pT[:], pT_ps[:])
                            for kc in range(NT):
                                nc.tensor.matmul(
                                    oT_ps[hs, qt * 128 : (qt + 1) * 128],
                                    v_sb[:, kc, hs],
                                    pT[:, kc, :],
                                    start=(kc == 0),
                                    stop=(kc == NT - 1),
                                )
                    nc.scalar.copy(oT_sb[:], oT_ps[:])

                    # ---- y partial = oT.T @ Wo -> DRAM (AllReduce input) ----
                    for t in range(NT):
                        y_ps = ps_w.tile([128, D], f32, tag="work", name=f"y_ps_{l}_{t}")
                        nc.tensor.matmul(
                            y_ps[:],
                            oT_sb[:, t * 128 : (t + 1) * 128],
                            wo_sb[:, l],
                            start=True,
                            stop=False,
                        )
                        nc.tensor.matmul(
                            y_ps[:],
                            ones_row[:, :128],
                            rows_row[:, 2 * L + l],
                            start=False,
                            stop=True,
                        )
                        y_sb = sb.tile([128, D], f32, tag="y_sb")
                        nc.scalar.copy(y_sb[:], y_ps[:])
                        nc.sync.dma_start(cc_in[l][t * 128 : (t + 1) * 128, :], y_sb[:])

                    if sim:
                        for t in range(NT):
                            cpt = sb.tile([128, D], f32, tag="cp", name=f"cp{_rep}_{l}_{t}")
                            nc.sync.dma_start(cpt[:], cc_in[l][t * 128 : (t + 1) * 128, :])
                            nc.sync.dma_start(cc_out[l][t * 128 : (t + 1) * 128, :], cpt[:])
                    else:
                        nc.gpsimd.collective_compute(
                            "AllReduce",
                            mybir.AluOpType.add,
                            replica_groups=GROUPS,
                            ins=[cc_in[l][:]],
                            outs=[cc_out[l][:]],
                        )

                    # ---- residual + LN (redundant on every core) ----
                    for t in range(NT):
                        yt = sb.tile([128, D], f32, tag="yt")
                        nc.sync.dma_start(yt[:], cc_out[l][t * 128 : (t + 1) * 128, :])
                        y1 = sb.tile([128, D], f32, tag="y1")
                        nc.vector.tensor_add(y1[:], yt[:], h_sb[:, t, :])
                        stats = sm.tile([128, 6], f32, tag="stats")
                        nc.vector.bn_stats(stats[:], y1[:])
                        mv = sm.tile([128, 2], f32, tag="mv")
                        nc.vector.bn_aggr(mv[:], stats[:])
                        std = sm.tile([128, 1], f32, tag="std")
                        nc.scalar.activation(
                            std[:], mv[:, 1:2], AF.Sqrt, scale=float(D / (D - 1))
                        )
                        rstd = sm.tile([128, 1], f32, tag="rstd")
                        nc.vector.tensor_scalar_add(std[:], std[:], float(EPS))
                        nc.vector.reciprocal(rstd[:], std[:])
                        hh = sb.tile([128, D], f32, tag="hh")
                        nc.vector.tensor_scalar(
                            hh[:],
                            y1[:],
                            mv[:, 0:1],
                            rstd[:],
                            op0=AL.subtract,
                            op1=AL.mult,
                        )
                        hn = sb.tile([128, D], f32, tag="hn")
                        nc.vector.tensor_mul(hn[:], hh[:], rows_bc[:, l])
                        nc.vector.tensor_add(hn[:], hn[:], rows_bc[:, L + l])
                        nc.vector.tensor_copy(h_sb[:, t, :], hn[:])
                        if l == L - 1:
                            nc.sync.dma_start(out_d[t * 128 : (t + 1) * 128, :], hn[:])

    nc.compile()
    return nc


def _get_compiled():
    global _COMPILED
    if _COMPILED is None:
        import os
        _COMPILED = _build(reps=int(os.environ.get("KERNEL_REPS", "1")))
    return _COMPILED


def _host_prep(x, Wq, Wk, Wv, Wo, bq, bk, bv, bo, gamma, beta):
    """Build the 8 per-core input maps."""
    Bv_Wo = np.stack([bv[l] @ Wo[l] + bo[l] for l in range(L)])  # [L, D]
    in_maps = []
    for c in range(8):
        b, r = divmod(c, 4)
        cols = slice(128 * r, 128 * (r + 1))
        rows = np.concatenate(
            [beta, gamma, Bv_Wo / 4.0], axis=0
        ).astype(np.float32)  # [3L, D]
        in_maps.append(
            {
                "x": np.ascontiguousarray(x[b]).astype(np.float32),
                "wq": np.ascontiguousarray(Wq[:, :, cols]).astype(np.float32),
                "wk": np.ascontiguousarray(Wk[:, :, cols]).astype(np.float32),
                "wv": np.ascontiguousarray(Wv[:, :, cols]).astype(np.float32),
                "wo": np.ascontiguousarray(Wo[:, cols, :]).astype(np.float32),
                "bq": np.ascontiguousarray(bq[:, cols]).astype(np.float32),
                "bk": np.ascontiguousarray(bk[:, cols]).astype(np.float32),
                "rows": rows,
            }
        )
    return in_maps


def _numpy_fallback(x, mask, Wq, Wk, Wv, Wo, bq, bk, bv, bo, gamma, beta):
    m = np.asarray(mask)[:, None, :, :]
    h = np.asarray(x, dtype=np.float64)
    for l in range(L):
        q = (h @ Wq[l] + bq[l]).reshape(B, S, H, DK).transpose(0, 2, 1, 3)
        k = (h @ Wk[l] + bk[l]).reshape(B, S, H, DK).transpose(0, 2, 1, 3)
        v = (h @ Wv[l] + bv[l]).reshape(B, S, H, DK).transpose(0, 2, 1, 3)
        s = np.einsum("bhqd,bhkd->bhqk", q, k) * SCALE
        kth = np.sort(s, axis=-1)[..., -TOPK][..., None]
        keep = (s >= kth) & m
        sm = np.where(keep, s, -1e9)
        sm = sm - sm.max(-1, keepdims=True)
        p = np.exp(sm)
        p /= p.sum(-1, keepdims=True)
        o = np.einsum("bhqk,bhkd->bhqd", p, v)
        o = o.transpose(0, 2, 1, 3).reshape(B, S, D) @ Wo[l] + bo[l]
        y = h + o
        mean = y.mean(-1, keepdims=True)
        std = y.std(-1, ddof=1, keepdims=True)
        h = beta[l] * (y - mean) / (std + EPS) + gamma[l]
    return h.astype(np.float32)


def kernel(x, mask, Wq, Wk, Wv, Wo, bq, bk, bv, bo, gamma, beta):
    x = np.asarray(x, dtype=np.float32)
    mask_np = np.asarray(mask)
    args = [np.asarray(a, dtype=np.float32) for a in (Wq, Wk, Wv, Wo, bq, bk, bv, bo, gamma, beta)]
    if not mask_np.all():
        return _numpy_fallback(x, mask_np, *args)

    from concourse import bass_utils

    nc = _get_compiled()
    in_maps = _host_prep(x, *args)
    res = bass_utils.run_bass_kernel_spmd(nc, in_maps, core_ids=list(range(8)))
    out = np.stack([res.results[0]["out"], res.results[4]["out"]])
    return out.astype(np.float32)

